# revision 11
# baseline (speedup 1.0000x reference)
"""Causal self-attention with RoPE on 8 Trainium2 NeuronCores.

Sharding: tensor-parallel over heads x data-parallel over batch.
  core c -> batch b = c // 2, head-group g = c % 2 (heads 8g .. 8g+7).
Each core computes qkv projections for its 8 heads, RoPE, causal
attention, and a *partial* output projection (its heads' contribution
to y[b]). Host sums the two partials per batch and adds the bias
terms (b_proj and the v-bias routed through W_proj).

v2: all matmuls in bf16 (inputs cast on host), everything SBUF-resident
(no DRAM bounce for q/k), rope via partition-shifted DVE muls (no PE
rotate matmuls), true-causal variable-width attention blocks, softmax
denominator broadcast via the Pool engine. Transposed orientation as in
v1: qT/kT [head_dim partitions, T free]; S^T [k part, q free]; O^T
[head_dim, q] which is exactly the lhsT layout phase C needs.
"""

import numpy as np
import ml_dtypes

import concourse.bass as bass
import concourse.mybir as mybir
import concourse.tile as tile
from concourse import bacc
from concourse.bass_utils import run_bass_kernel_spmd

F32 = mybir.dt.float32
BF16 = mybir.dt.bfloat16
AF = mybir.ActivationFunctionType
ALU = mybir.AluOpType

D_MODEL = 2048
N_HEADS = 16
HD = 128
B, T = 4, 2048
N_CORES = 8
HPC = 8           # heads per core
PB = 128          # partitions / k-chunk
XSL = 512         # x^T t-slice width in phase A
QB = 512          # phase-B query-chunk width (one PSUM bank of f32)
EC = 512          # phase-C output-column chunk width
SCALE = 1.0 / np.sqrt(HD)

BF = ml_dtypes.bfloat16


def build_nc(t=T, d=D_MODEL, hpc=HPC, compile=True):
    """Build the per-core Bass module. All 8 cores run this same module on
    different input slices."""
    nc = bacc.Bacc(trn_type="TRN2", target_bir_lowering=False)

    dck = d // PB          # D-chunks (contraction tiles)
    nsl = t // XSL         # phase-A t-slices
    ntc128 = t // PB       # t-chunks of 128
    hw = hpc * HD          # this core's head width
    nqc = t // QB          # phase-B query chunks
    kpq = QB // PB         # k-blocks per query chunk

    xT = nc.dram_tensor("xT", [d, t], BF16, kind="ExternalInput")
    wq = nc.dram_tensor("wq", [d, hw], BF16, kind="ExternalInput")
    wk = nc.dram_tensor("wk", [d, hw], BF16, kind="ExternalInput")
    wv = nc.dram_tensor("wv", [d, hw], BF16, kind="ExternalInput")
    bq = nc.dram_tensor("bq", [hw], F32, kind="ExternalInput")
    bk = nc.dram_tensor("bk", [hw], F32, kind="ExternalInput")
    wp = nc.dram_tensor("wp", [hw, d], BF16, kind="ExternalInput")
    cosT = nc.dram_tensor("cosT", [HD, t], BF16, kind="ExternalInput")
    # sinTr is sign-folded (rows 0:64 of the raw table negated) and then
    # rolled by 64 partitions, so the partition-shifted rope muls read both
    # DVE inputs at the same base partition (walrus requirement).
    sinTr = nc.dram_tensor("sinTr", [HD, t], BF16, kind="ExternalInput")
    y = nc.dram_tensor("y", [t, d], F32, kind="ExternalOutput")

    with tile.TileContext(nc) as tc:
        with (
            tc.tile_pool(name="consts", bufs=1) as consts,
            tc.tile_pool(name="qkres", bufs=1) as qkres,
        ):
            # --- constants -------------------------------------------------
            ones_f = consts.tile([PB, 1], F32, tag="ones_f")
            nc.vector.memset(ones_f, 1.0)
            ones_col = consts.tile([PB, 1], BF16, tag="ones")
            nc.vector.tensor_copy(ones_col, ones_f)
            # causal masks for the 4 diagonal k-block positions within a
            # 512-wide q-chunk: masks[r][p, f] = 1.0 iff f >= p + 128*r
            masks = []
            for r in range(QB // PB):
                mk_f = consts.tile([PB, QB], F32, tag=f"mask_f{r}",
                                   name=f"mask_f{r}")
                nc.vector.memset(mk_f, 1.0)
                nc.gpsimd.affine_select(
                    out=mk_f, in_=mk_f, compare_op=ALU.is_ge, fill=0.0,
                    base=-(r * PB), pattern=[[1, QB]], channel_multiplier=-1,
                )
                mk = consts.tile([PB, QB], BF16, tag=f"mask{r}",
                                 name=f"mask{r}")
                nc.vector.tensor_copy(mk, mk_f)
                masks.append(mk)

            # persistent across phases
            qT_all = qkres.tile([HD, hpc, t], BF16, tag="qT")
            kT_all = qkres.tile([HD, hpc, t], BF16, tag="kT")

            x_src = xT.ap().rearrange("(c p) t -> p c t", p=PB)

            # ================= Phase A1: q,k projections + RoPE ===========
            with (
                tc.tile_pool(name="ropec", bufs=1) as ropec,
                tc.tile_pool(name="w_qk", bufs=1) as pw,
                tc.tile_pool(name="x_a1", bufs=3) as px,
                tc.tile_pool(name="t_a1", bufs=3) as pt,
                tc.tile_pool(name="psA1", bufs=3, space="PSUM") as psA,
            ):
                cosT_s = ropec.tile([HD, t], BF16, tag="cosT")
                sinT_s = ropec.tile([HD, t], BF16, tag="sinT")
                bq_s = ropec.tile([HD, hpc], F32, tag="bq")
                bk_s = ropec.tile([HD, hpc], F32, tag="bk")

                wq_s = pw.tile([PB, dck, hw], BF16, tag="wq", name="wq_s")
                wk_s = pw.tile([PB, dck, hw], BF16, tag="wk", name="wk_s")
                wq_src = wq.ap().rearrange("(c p) m -> p c m", p=PB)
                wk_src = wk.ap().rearrange("(c p) m -> p c m", p=PB)
                # issue order = consumption order: head-0 q-weights, first x
                # slice, rope tables, then the rest.
                nc.sync.dma_start(out=wq_s[:, :, 0:HD], in_=wq_src[:, :, 0:HD])
                xt0 = px.tile([PB, dck, XSL], BF16, tag="xt", name="xt_s")
                for cc in range(0, dck, 4):
                    nc.sync.dma_start(
                        out=xt0[:, cc:cc + 4, :], in_=x_src[:, cc:cc + 4, 0:XSL]
                    )
                nc.sync.dma_start(out=cosT_s, in_=cosT.ap())
                nc.sync.dma_start(out=sinT_s, in_=sinTr.ap())
                nc.sync.dma_start(
                    out=bq_s, in_=bq.ap().rearrange("(h p) -> p h", p=HD)
                )
                nc.sync.dma_start(
                    out=bk_s, in_=bk.ap().rearrange("(h p) -> p h", p=HD)
                )
                for h in range(1, hpc):
                    hs = slice(h * HD, (h + 1) * HD)
                    nc.sync.dma_start(out=wq_s[:, :, hs], in_=wq_src[:, :, hs])
                for h in range(hpc):
                    hs = slice(h * HD, (h + 1) * HD)
                    nc.sync.dma_start(out=wk_s[:, :, hs], in_=wk_src[:, :, hs])

                for sl in range(nsl):
                    ts = slice(sl * XSL, (sl + 1) * XSL)
                    if sl == 0:
                        xt_s = xt0
                    else:
                        xt_s = px.tile([PB, dck, XSL], BF16, tag="xt",
                                       name="xt_s")
                        for cc in range(0, dck, 4):
                            nc.sync.dma_start(
                                out=xt_s[:, cc:cc + 4, :],
                                in_=x_src[:, cc:cc + 4, ts],
                            )
                    for w_s, bias_s, outT in (
                        (wq_s, bq_s, qT_all),
                        (wk_s, bk_s, kT_all),
                    ):
                        for h in range(hpc):
                            ps = psA.tile([PB, XSL], F32, tag="ps_a",
                                          name="ps_a")
                            for c in range(dck):
                                nc.tensor.matmul(
                                    ps,
                                    lhsT=w_s[:, c, h * HD:(h + 1) * HD],
                                    rhs=xt_s[:, c, :],
                                    start=(c == 0),
                                    stop=(c == dck - 1),
                                )
                            # evict + per-partition bias on ACT -> bf16
                            raw = pt.tile([PB, XSL], BF16, tag="raw",
                                          name="raw")
                            nc.scalar.activation(
                                out=raw, in_=ps, func=AF.Identity,
                                bias=bias_s[:, h:h + 1], scale=1.0,
                            )
                            # rope on DVE: out = raw*cos + shift64(raw)*sins
                            rs = pt.tile([PB, XSL], BF16, tag="rs", name="rs")
                            nc.vector.tensor_mul(
                                rs[0:64], raw[64:128], sinT_s[64:128, ts])
                            nc.vector.tensor_mul(
                                rs[64:128], raw[0:64], sinT_s[0:64, ts])
                            cq = pt.tile([PB, XSL], BF16, tag="cq", name="cq")
                            nc.vector.tensor_mul(cq, raw, cosT_s[:, ts])
                            nc.vector.tensor_add(outT[:, h, ts], cq, rs)

            # ================= Phase A2: v projection =====================
            # v natural layout (lhsT=x^T tile, rhs=W cols); v stays resident.
            with tc.tile_pool(name="vall", bufs=1) as pv:
                v_all = pv.tile([PB, ntc128, hw], BF16, tag="v_all",
                                name="v_all")
                with (
                    tc.tile_pool(name="w_v", bufs=1) as pwv,
                    tc.tile_pool(name="x_a2", bufs=3) as px,
                    tc.tile_pool(name="psA2", bufs=2, space="PSUM") as psA,
                ):
                    wv_s = pwv.tile([PB, dck, hw], BF16, tag="wv", name="wv_s")
                    wv_src = wv.ap().rearrange("(c p) m -> p c m", p=PB)
                    xt0v = px.tile([PB, dck, XSL], BF16, tag="xt", name="xt_s")
                    for cc in range(0, dck, 4):
                        nc.sync.dma_start(
                            out=xt0v[:, cc:cc + 4, :],
                            in_=x_src[:, cc:cc + 4, 0:XSL],
                        )
                        nc.sync.dma_start(
                            out=wv_s[:, cc:cc + 4, :],
                            in_=wv_src[:, cc:cc + 4, :],
                        )
                    for sl in range(nsl):
                        ts = slice(sl * XSL, (sl + 1) * XSL)
                        if sl == 0:
                            xt_s = xt0v
                        else:
                            xt_s = px.tile([PB, dck, XSL], BF16, tag="xt",
                                           name="xt_s")
                            for cc in range(0, dck, 4):
                                nc.sync.dma_start(
                                    out=xt_s[:, cc:cc + 4, :],
                                    in_=x_src[:, cc:cc + 4, ts],
                                )
                        for t128 in range(XSL // PB):
                            kcg = sl * (XSL // PB) + t128
                            for nci in range(hw // 512):
                                ns = slice(nci * 512, (nci + 1) * 512)
                                ps = psA.tile([PB, 512], F32, tag="ps_v",
                                              name="ps_v")
                                for c in range(dck):
                                    nc.tensor.matmul(
                                        ps,
                                        lhsT=xt_s[:, c,
                                                  t128 * PB:(t128 + 1) * PB],
                                        rhs=wv_s[:, c, ns],
                                        start=(c == 0),
                                        stop=(c == dck - 1),
                                    )
                                nc.vector.tensor_copy(
                                    v_all[:, kcg, ns], ps)

                # =============== Phase B: attention per head ==============
                with (
                    tc.tile_pool(name="ot", bufs=1) as po,
                    tc.tile_pool(name="wp_p", bufs=1) as pwp,
                ):
                    ot_all = po.tile([HD, hpc, t], BF16, tag="ot")
                    # prefetch the output-projection weights during phase B
                    wp_s = pwp.tile([PB, hpc, d], BF16, tag="wp", name="wp_s")
                    wp_src = wp.ap().rearrange("(h p) e -> p h e", p=PB)
                    for h in range(hpc):
                        nc.sync.dma_start(
                            out=wp_s[:, h, :], in_=wp_src[:, h, :]
                        )
                    with (
                        tc.tile_pool(name="pt_pool", bufs=3) as pp,
                        tc.tile_pool(name="small", bufs=2) as psm,
                        tc.tile_pool(name="psB", bufs=1, space="PSUM") as psB,
                    ):
                        # flat list of k-block PAIRS; software-pipelined:
                        # the S matmuls of pair i+1 are issued on the PE
                        # queue before z/O of pair i, so the PE computes the
                        # next score pair while ACT runs exp(i). Matmul PSUM
                        # outputs are one bank (512 f32) wide - ISA limit.
                        pairs = []
                        for h in range(hpc):
                            for qc in range(nqc):
                                for kcp in range((qc + 1) * kpq // 2):
                                    pairs.append((h, qc, kcp))
                        st = {}

                        def emit_S(i):
                            h, qc, kcp = pairs[i]
                            if kcp == 0:
                                st[(h, qc, "o")] = psB.tile(
                                    [HD, QB], F32, tag="ps_o", name="ps_o",
                                    bufs=2)
                                st[(h, qc, "z")] = psB.tile(
                                    [1, QB], F32, tag="ps_z", name="ps_z",
                                    bufs=2)
                            ps_s2 = psB.tile([PB, 2 * QB], F32, tag="ps_s",
                                             name="ps_s2", bufs=2)
                            qs = slice(qc * QB, (qc + 1) * QB)
                            for j in (0, 1):
                                kc = 2 * kcp + j
                                nc.tensor.matmul(
                                    ps_s2[:, j * QB:(j + 1) * QB],
                                    lhsT=kT_all[:, h, kc * PB:(kc + 1) * PB],
                                    rhs=qT_all[:, h, qs],
                                    start=True, stop=True,
                                )
                            st[i] = ps_s2

                        def emit_expmask(i):
                            h, qc, kcp = pairs[i]
                            ps_s2 = st.pop(i)
                            pt2 = pp.tile([PB, 2 * QB], BF16, tag="pt",
                                          name="pt2")
                            nc.scalar.activation(
                                out=pt2, in_=ps_s2, func=AF.Exp, scale=SCALE,
                            )
                            for j in (0, 1):
                                r = 2 * kcp + j - qc * kpq
                                if r >= 0:
                                    nc.vector.tensor_mul(
                                        pt2[:, j * QB:(j + 1) * QB],
                                        pt2[:, j * QB:(j + 1) * QB],
                                        masks[r],
                                    )
                            st[(i, "pt")] = pt2

                        def emit_zo(i):
                            h, qc, kcp = pairs[i]
                            nkc = (qc + 1) * kpq
                            pt2 = st.pop((i, "pt"))
                            ps_z = st[(h, qc, "z")]
                            ps_o = st[(h, qc, "o")]
                            for j in (0, 1):
                                kc = 2 * kcp + j
                                pt_t = pt2[:, j * QB:(j + 1) * QB]
                                nc.tensor.matmul(
                                    ps_z,
                                    lhsT=ones_col,
                                    rhs=pt_t,
                                    start=(kc == 0), stop=(kc == nkc - 1),
                                )
                                nc.tensor.matmul(
                                    ps_o,
                                    lhsT=v_all[:, kc, h * HD:(h + 1) * HD],
                                    rhs=pt_t,
                                    start=(kc == 0), stop=(kc == nkc - 1),
                                )
                            if 2 * kcp + 1 == nkc - 1:
                                # normalize: ot = ps_o * (1/z) broadcast
                                qs = slice(qc * QB, (qc + 1) * QB)
                                rz = psm.tile([1, QB], F32, tag="rz",
                                              name="rz")
                                nc.vector.reciprocal_approx_fast(
                                    out=rz, in_=st.pop((h, qc, "z")))
                                rzb = pp.tile([HD, QB], F32, tag="rzb",
                                              name="rzb", bufs=2)
                                nc.gpsimd.partition_broadcast(rzb, rz)
                                ot_tmp = pp.tile([HD, QB], BF16,
                                                 tag="ot_tmp", name="ot_tmp",
                                                 bufs=2)
                                nc.vector.tensor_copy(
                                    ot_tmp, st.pop((h, qc, "o")))
                                nc.vector.tensor_mul(
                                    ot_all[:, h, qs], ot_tmp, rzb)

                        emit_S(0)
                        for i in range(len(pairs)):
                            emit_expmask(i)
                            if i + 1 < len(pairs):
                                emit_S(i + 1)
                            emit_zo(i)

                    # ================= Phase C: output projection =========
                    with (
                        tc.tile_pool(name="yout", bufs=3) as py,
                        tc.tile_pool(name="psC", bufs=2, space="PSUM") as psC,
                    ):
                        for nci in range(d // EC):
                            es = slice(nci * EC, (nci + 1) * EC)
                            for t128 in range(ntc128):
                                ps_y = psC.tile([PB, EC], F32, tag="ps_y",
                                                name="ps_y", bufs=3)
                                for h in range(hpc):
                                    nc.tensor.matmul(
                                        ps_y,
                                        lhsT=ot_all[:, h,
                                                    t128 * PB:(t128 + 1) * PB],
                                        rhs=wp_s[:, h, es],
                                        start=(h == 0), stop=(h == hpc - 1),
                                    )
                                y_t = py.tile([PB, EC], F32, tag="y_t",
                                              name="y_t")
                                if t128 % 2 == 0:
                                    nc.vector.tensor_copy(y_t, ps_y)
                                else:
                                    nc.scalar.copy(y_t, ps_y)
                                nc.sync.dma_start(
                                    out=y.ap()[t128 * PB:(t128 + 1) * PB, es],
                                    in_=y_t,
                                )
    if compile:
        nc.compile()
    return nc


def make_in_maps(x, cos, sin, W_qkv, b_qkv, W_proj):
    """Host-side sharding: build the 8 per-core input dicts (bf16 casts)."""
    d = x.shape[-1]
    in_maps = []
    cosT = np.ascontiguousarray(cos.reshape(-1, HD).T).astype(np.float32)
    sinT = np.ascontiguousarray(sin.reshape(-1, HD).T).astype(np.float32)
    sinTs = sinT.copy()
    sinTs[: HD // 2] = -sinTs[: HD // 2]
    sinTr = np.roll(sinTs, -(HD // 2), axis=0)
    cosT = cosT.astype(BF)
    sinTr = sinTr.astype(BF)
    Wq = np.asarray(W_qkv[:, 0 * d:1 * d], np.float32)
    Wk = np.asarray(W_qkv[:, 1 * d:2 * d], np.float32)
    Wv = np.asarray(W_qkv[:, 2 * d:3 * d], np.float32)
    for c in range(N_CORES):
        b = c // 2
        g = c % 2
        hw = HPC * HD
        cs = slice(g * hw, (g + 1) * hw)
        in_maps.append(
            {
                "xT": np.ascontiguousarray(
                    np.asarray(x[b], np.float32).T).astype(BF),
                "wq": np.ascontiguousarray(Wq[:, cs]).astype(BF),
                "wk": np.ascontiguousarray(Wk[:, cs]).astype(BF),
                "wv": np.ascontiguousarray(Wv[:, cs]).astype(BF),
                "bq": np.ascontiguousarray(b_qkv[0 * d:1 * d][cs], np.float32),
                "bk": np.ascontiguousarray(b_qkv[1 * d:2 * d][cs], np.float32),
                "wp": np.ascontiguousarray(
                    np.asarray(W_proj, np.float32)[g * hw:(g + 1) * hw, :]
                ).astype(BF),
                "cosT": cosT,
                "sinTr": sinTr,
            }
        )
    return in_maps


def gather_output(results, b_qkv, W_proj, b_proj):
    """Sum the per-core partials and add the bias terms."""
    d = W_proj.shape[1]
    # v-bias contributes (sum_k attn = 1) exactly b_v @ W_proj per token.
    host_bias = (
        np.asarray(b_qkv[2 * d:3 * d], np.float32)
        @ np.asarray(W_proj, np.float32)
        + np.asarray(b_proj, np.float32)
    )
    y = np.empty((B, T, d), np.float32)
    for b in range(B):
        y[b] = results[2 * b]["y"] + results[2 * b + 1]["y"] + host_bias
    return y


_NC_CACHE = {}


def kernel(x, cos, sin, W_qkv, b_qkv, W_proj, b_proj):
    x = np.asarray(x, np.float32)
    key = "full"
    if key not in _NC_CACHE:
        _NC_CACHE[key] = build_nc()
    nc = _NC_CACHE[key]
    in_maps = make_in_maps(
        x,
        np.asarray(cos, np.float32),
        np.asarray(sin, np.float32),
        np.asarray(W_qkv, np.float32),
        np.asarray(b_qkv, np.float32),
        np.asarray(W_proj, np.float32),
    )
    res = run_bass_kernel_spmd(nc, in_maps, core_ids=list(range(N_CORES)))
    return gather_output(res.results, b_qkv, W_proj, b_proj)


if __name__ == "__main__":
    import reference

    inputs = reference.setup_inputs()
    out = kernel(**{k: np.asarray(v) for k, v in inputs.items()})
    exp = np.asarray(reference.reference(**inputs))
    err = np.abs(out - exp).max() / np.abs(exp).max()
    print("rel err:", err)


# revision 16
# speedup vs baseline: 1.0282x; 1.0282x over previous
"""Causal self-attention with RoPE on 8 Trainium2 NeuronCores.

Sharding: tensor-parallel over heads x data-parallel over batch.
  core c -> batch b = c // 2, head-group g = c % 2 (heads 8g .. 8g+7).
Each core computes qkv projections for its 8 heads, RoPE, causal
attention, and a *partial* output projection (its heads' contribution
to y[b]). Host sums the two partials per batch and adds the bias
terms (b_proj and the v-bias routed through W_proj).

v2: all matmuls in bf16 (inputs cast on host), everything SBUF-resident
(no DRAM bounce for q/k), rope via partition-shifted DVE muls (no PE
rotate matmuls), true-causal variable-width attention blocks, softmax
denominator broadcast via the Pool engine. Transposed orientation as in
v1: qT/kT [head_dim partitions, T free]; S^T [k part, q free]; O^T
[head_dim, q] which is exactly the lhsT layout phase C needs.
"""

import numpy as np
import ml_dtypes

import concourse.bass as bass
import concourse.mybir as mybir
import concourse.tile as tile
from concourse import bacc
from concourse.bass_utils import run_bass_kernel_spmd

F32 = mybir.dt.float32
BF16 = mybir.dt.bfloat16
AF = mybir.ActivationFunctionType
ALU = mybir.AluOpType

D_MODEL = 2048
N_HEADS = 16
HD = 128
B, T = 4, 2048
N_CORES = 8
HPC = 8           # heads per core
PB = 128          # partitions / k-chunk
XSL = 512         # x^T t-slice width in phase A
QB = 512          # phase-B query-chunk width (one PSUM bank of f32)
EC = 512          # phase-C output-column chunk width
SCALE = 1.0 / np.sqrt(HD)

BF = ml_dtypes.bfloat16


def build_nc(t=T, d=D_MODEL, hpc=HPC, compile=True):
    """Build the per-core Bass module. All 8 cores run this same module on
    different input slices."""
    nc = bacc.Bacc(trn_type="TRN2", target_bir_lowering=False)

    dck = d // PB          # D-chunks (contraction tiles)
    nsl = t // XSL         # phase-A t-slices
    ntc128 = t // PB       # t-chunks of 128
    hw = hpc * HD          # this core's head width
    nqc = t // QB          # phase-B query chunks
    kpq = QB // PB         # k-blocks per query chunk

    xT = nc.dram_tensor("xT", [d, t], BF16, kind="ExternalInput")
    wq = nc.dram_tensor("wq", [d, hw], BF16, kind="ExternalInput")
    wk = nc.dram_tensor("wk", [d, hw], BF16, kind="ExternalInput")
    wv = nc.dram_tensor("wv", [d, hw], BF16, kind="ExternalInput")
    bq = nc.dram_tensor("bq", [hw], F32, kind="ExternalInput")
    bk = nc.dram_tensor("bk", [hw], F32, kind="ExternalInput")
    wp = nc.dram_tensor("wp", [hw, d], BF16, kind="ExternalInput")
    cosT = nc.dram_tensor("cosT", [HD, t], BF16, kind="ExternalInput")
    # sinTr is sign-folded (rows 0:64 of the raw table negated) and then
    # rolled by 64 partitions, so the partition-shifted rope muls read both
    # DVE inputs at the same base partition (walrus requirement).
    sinTr = nc.dram_tensor("sinTr", [HD, t], BF16, kind="ExternalInput")
    y = nc.dram_tensor("y", [t, d], F32, kind="ExternalOutput")

    with tile.TileContext(nc) as tc:
        with (
            tc.tile_pool(name="consts", bufs=1) as consts,
            tc.tile_pool(name="qkres", bufs=1) as qkres,
            tc.tile_pool(name="vall", bufs=1) as pv,
        ):
            # --- constants -------------------------------------------------
            ones_f = consts.tile([PB, 1], F32, tag="ones_f")
            nc.vector.memset(ones_f, 1.0)
            ones_col = consts.tile([PB, 1], BF16, tag="ones")
            nc.vector.tensor_copy(ones_col, ones_f)
            # causal masks for the 4 diagonal k-block positions within a
            # 512-wide q-chunk: masks[r][p, f] = 1.0 iff f >= p + 128*r
            masks = []
            with tc.tile_pool(name="mask_scratch", bufs=1) as msc:
                for r in range(QB // PB):
                    mk_f = msc.tile([PB, QB], F32, tag=f"mask_f{r}",
                                    name=f"mask_f{r}")
                    nc.vector.memset(mk_f, 1.0)
                    nc.gpsimd.affine_select(
                        out=mk_f, in_=mk_f, compare_op=ALU.is_ge, fill=0.0,
                        base=-(r * PB), pattern=[[1, QB]],
                        channel_multiplier=-1,
                    )
                    mk = consts.tile([PB, QB], BF16, tag=f"mask{r}",
                                     name=f"mask{r}")
                    nc.vector.tensor_copy(mk, mk_f)
                    masks.append(mk)

            # persistent tiles: q^T/k^T (through B) and v (through B)
            qT_all = qkres.tile([HD, hpc, t], BF16, tag="qT")
            kT_all = qkres.tile([HD, hpc, t], BF16, tag="kT")
            v_all = pv.tile([PB, ntc128, hw], BF16, tag="v_all", name="v_all")

            x_src = xT.ap().rearrange("(c p) t -> p c t", p=PB)

            # ============ Phase A1+A2: projections (x fully resident) ======
            with tc.tile_pool(name="x_a", bufs=4) as px:
                xt_tiles = []
                # ---- A1: q,k per head (weights streamed per head) + RoPE --
                with (
                    tc.tile_pool(name="ropec", bufs=1) as ropec,
                    tc.tile_pool(name="w_qk", bufs=4) as pw,
                    tc.tile_pool(name="t_a1", bufs=2) as pt,
                    tc.tile_pool(name="psA1", bufs=3, space="PSUM") as psA,
                ):
                    cosT_s = ropec.tile([HD, t], BF16, tag="cosT")
                    sinT_s = ropec.tile([HD, t], BF16, tag="sinT")
                    bq_s = ropec.tile([HD, hpc], F32, tag="bq")
                    bk_s = ropec.tile([HD, hpc], F32, tag="bk")

                    wq_src = wq.ap().rearrange("(c p) m -> p c m", p=PB)
                    wk_src = wk.ap().rearrange("(c p) m -> p c m", p=PB)

                    # DMA order = consumption order: x slice 0, head-0
                    # q/k weights, remaining x slices, then per-head weights.
                    w_tiles = {}

                    def load_w(h):
                        hs = slice(h * HD, (h + 1) * HD)
                        for kind, w_src in (("q", wq_src), ("k", wk_src)):
                            w_h = pw.tile([PB, dck, HD], BF16, tag="w",
                                          name="w_h")
                            nc.sync.dma_start(out=w_h, in_=w_src[:, :, hs])
                            w_tiles[(kind, h)] = w_h

                    for sl in range(nsl):
                        xt_s = px.tile([PB, dck, XSL], BF16, tag="xt",
                                       name="xt_s")
                        xt_tiles.append(xt_s)
                        for cc in range(0, dck, 4):
                            nc.sync.dma_start(
                                out=xt_s[:, cc:cc + 4, :],
                                in_=x_src[:, cc:cc + 4,
                                          sl * XSL:(sl + 1) * XSL],
                            )
                        if sl == 0:
                            nc.sync.dma_start(out=cosT_s, in_=cosT.ap())
                            nc.sync.dma_start(out=sinT_s, in_=sinTr.ap())
                            nc.sync.dma_start(
                                out=bq_s,
                                in_=bq.ap().rearrange("(h p) -> p h", p=HD))
                            nc.sync.dma_start(
                                out=bk_s,
                                in_=bk.ap().rearrange("(h p) -> p h", p=HD))
                            load_w(0)
                    for h in range(1, hpc):
                        load_w(h)

                    for h in range(hpc):
                        for sl in range(nsl):
                            ts = slice(sl * XSL, (sl + 1) * XSL)
                            xt_s = xt_tiles[sl]
                            for kind, bias_s, outT in (
                                ("q", bq_s, qT_all),
                                ("k", bk_s, kT_all),
                            ):
                                w_h = w_tiles[(kind, h)]
                                ps = psA.tile([PB, XSL], F32, tag="ps_a",
                                              name="ps_a")
                                for c in range(dck):
                                    nc.tensor.matmul(
                                        ps,
                                        lhsT=w_h[:, c, :],
                                        rhs=xt_s[:, c, :],
                                        start=(c == 0),
                                        stop=(c == dck - 1),
                                    )
                                # evict + per-partition bias on ACT -> bf16
                                raw = pt.tile([PB, XSL], BF16, tag="raw",
                                              name="raw")
                                nc.scalar.activation(
                                    out=raw, in_=ps, func=AF.Identity,
                                    bias=bias_s[:, h:h + 1], scale=1.0,
                                )
                                # rope on DVE:
                                #   out = raw*cos + shift64(raw)*sin_folded
                                rs = pt.tile([PB, XSL], BF16, tag="rs",
                                             name="rs")
                                nc.vector.tensor_mul(
                                    rs[0:64], raw[64:128],
                                    sinT_s[64:128, ts])
                                nc.vector.tensor_mul(
                                    rs[64:128], raw[0:64], sinT_s[0:64, ts])
                                cq = pt.tile([PB, XSL], BF16, tag="cq",
                                             name="cq")
                                nc.vector.tensor_mul(cq, raw, cosT_s[:, ts])
                                nc.vector.tensor_add(outT[:, h, ts], cq, rs)
                        for kind in ("q", "k"):
                            w_tiles.pop((kind, h))

                # ---- A2: v projection (natural layout, x slices reused) ---
                # nci-outer: the first half of wv covers 16 groups of
                # compute, so the second half streams in fully hidden.
                with (
                    tc.tile_pool(name="w_v", bufs=1) as pwv,
                    tc.tile_pool(name="psA2", bufs=3, space="PSUM") as psA,
                ):
                    wv_s = pwv.tile([PB, dck, hw], BF16, tag="wv", name="wv_s")
                    wv_src = wv.ap().rearrange("(c p) m -> p c m", p=PB)
                    for nci in range(hw // 512):
                        ns = slice(nci * 512, (nci + 1) * 512)
                        for cc in range(0, dck, 4):
                            nc.sync.dma_start(
                                out=wv_s[:, cc:cc + 4, ns],
                                in_=wv_src[:, cc:cc + 4, ns],
                            )
                    for nci in range(hw // 512):
                        ns = slice(nci * 512, (nci + 1) * 512)
                        for kcg in range(ntc128):
                            xt_s = xt_tiles[kcg // (XSL // PB)]
                            t128 = kcg % (XSL // PB)
                            ps = psA.tile([PB, 512], F32, tag="ps_v",
                                          name="ps_v")
                            for c in range(dck):
                                nc.tensor.matmul(
                                    ps,
                                    lhsT=xt_s[:, c,
                                              t128 * PB:(t128 + 1) * PB],
                                    rhs=wv_s[:, c, ns],
                                    start=(c == 0),
                                    stop=(c == dck - 1),
                                )
                            nc.vector.tensor_copy(v_all[:, kcg, ns], ps)

            # =============== Phase B: attention per head ===================
            with (
                tc.tile_pool(name="ot", bufs=1) as po,
                tc.tile_pool(name="wp_p", bufs=1) as pwp,
            ):
                ot_all = po.tile([HD, hpc, t], BF16, tag="ot")
                # prefetch the output-projection weights during phase B
                wp_s = pwp.tile([PB, hpc, d], BF16, tag="wp", name="wp_s")
                wp_src = wp.ap().rearrange("(h p) e -> p h e", p=PB)
                for h in range(hpc):
                    nc.sync.dma_start(out=wp_s[:, h, :], in_=wp_src[:, h, :])
                with (
                    tc.tile_pool(name="pt_pool", bufs=3) as pp,
                    tc.tile_pool(name="small", bufs=2) as psm,
                    tc.tile_pool(name="psB", bufs=1, space="PSUM") as psB,
                ):
                    # flat list of k-block PAIRS, software-pipelined at
                    # depth 2: the S matmuls of pairs i+1, i+2 are issued on
                    # the PE queue before z/O of pair i, so the PE is never
                    # waiting on ACT's exp. Matmul PSUM outputs are one bank
                    # (512 f32) wide - ISA limit.
                    pairs = []
                    for h in range(hpc):
                        for qc in range(nqc):
                            for kcp in range((qc + 1) * kpq // 2):
                                pairs.append((h, qc, kcp))
                    st = {}

                    def emit_S(i):
                        h, qc, kcp = pairs[i]
                        if kcp == 0:
                            st[(h, qc, "o")] = psB.tile(
                                [HD, QB], F32, tag="ps_o", name="ps_o",
                                bufs=1)
                            st[(h, qc, "z")] = psB.tile(
                                [1, QB], F32, tag="ps_z", name="ps_z",
                                bufs=1)
                        ps_s2 = psB.tile([PB, 2 * QB], F32, tag="ps_s",
                                         name="ps_s2", bufs=3)
                        qs = slice(qc * QB, (qc + 1) * QB)
                        for j in (0, 1):
                            kc = 2 * kcp + j
                            nc.tensor.matmul(
                                ps_s2[:, j * QB:(j + 1) * QB],
                                lhsT=kT_all[:, h, kc * PB:(kc + 1) * PB],
                                rhs=qT_all[:, h, qs],
                                start=True, stop=True,
                            )
                        st[i] = ps_s2

                    def emit_expmask(i):
                        h, qc, kcp = pairs[i]
                        ps_s2 = st.pop(i)
                        pt2 = pp.tile([PB, 2 * QB], BF16, tag="pt",
                                      name="pt2")
                        nc.scalar.activation(
                            out=pt2, in_=ps_s2, func=AF.Exp, scale=SCALE,
                        )
                        for j in (0, 1):
                            r = 2 * kcp + j - qc * kpq
                            if r >= 0:
                                nc.vector.tensor_mul(
                                    pt2[:, j * QB:(j + 1) * QB],
                                    pt2[:, j * QB:(j + 1) * QB],
                                    masks[r],
                                )
                        st[(i, "pt")] = pt2

                    def emit_zo(i):
                        h, qc, kcp = pairs[i]
                        nkc = (qc + 1) * kpq
                        pt2 = st.pop((i, "pt"))
                        ps_z = st[(h, qc, "z")]
                        ps_o = st[(h, qc, "o")]
                        for j in (0, 1):
                            kc = 2 * kcp + j
                            pt_t = pt2[:, j * QB:(j + 1) * QB]
                            nc.tensor.matmul(
                                ps_z,
                                lhsT=ones_col,
                                rhs=pt_t,
                                start=(kc == 0), stop=(kc == nkc - 1),
                            )
                            nc.tensor.matmul(
                                ps_o,
                                lhsT=v_all[:, kc, h * HD:(h + 1) * HD],
                                rhs=pt_t,
                                start=(kc == 0), stop=(kc == nkc - 1),
                            )
                        if 2 * kcp + 1 == nkc - 1:
                            # normalize: ot = ps_o * (1/z) broadcast. The
                            # ps_o eviction is issued first so the single
                            # ps_o buffer frees before the next chunk's O.
                            qs = slice(qc * QB, (qc + 1) * QB)
                            ot_tmp = pp.tile([HD, QB], BF16,
                                             tag="ot_tmp", name="ot_tmp",
                                             bufs=2)
                            nc.vector.tensor_copy(
                                ot_tmp, st.pop((h, qc, "o")))
                            rz = psm.tile([1, QB], F32, tag="rz", name="rz")
                            nc.vector.reciprocal_approx_fast(
                                out=rz, in_=st.pop((h, qc, "z")))
                            rzb = pp.tile([HD, QB], F32, tag="rzb",
                                          name="rzb", bufs=2)
                            nc.gpsimd.partition_broadcast(rzb, rz)
                            nc.vector.tensor_mul(
                                ot_all[:, h, qs], ot_tmp, rzb)

                    emit_S(0)
                    emit_expmask(0)
                    emit_S(1)
                    for i in range(len(pairs)):
                        if i + 2 < len(pairs):
                            emit_S(i + 2)
                        if i + 1 < len(pairs):
                            emit_expmask(i + 1)
                        emit_zo(i)

                # ================= Phase C: output projection ==============
                with (
                    tc.tile_pool(name="yout", bufs=3) as py,
                    tc.tile_pool(name="psC", bufs=2, space="PSUM") as psC,
                ):
                    for nci in range(d // EC):
                        es = slice(nci * EC, (nci + 1) * EC)
                        for t128 in range(ntc128):
                            ps_y = psC.tile([PB, EC], F32, tag="ps_y",
                                            name="ps_y", bufs=3)
                            for h in range(hpc):
                                nc.tensor.matmul(
                                    ps_y,
                                    lhsT=ot_all[:, h,
                                                t128 * PB:(t128 + 1) * PB],
                                    rhs=wp_s[:, h, es],
                                    start=(h == 0), stop=(h == hpc - 1),
                                )
                            y_t = py.tile([PB, EC], F32, tag="y_t",
                                          name="y_t")
                            if t128 % 2 == 0:
                                nc.vector.tensor_copy(y_t, ps_y)
                            else:
                                nc.scalar.copy(y_t, ps_y)
                            nc.sync.dma_start(
                                out=y.ap()[t128 * PB:(t128 + 1) * PB, es],
                                in_=y_t,
                            )
    if compile:
        nc.compile()
    return nc


def make_in_maps(x, cos, sin, W_qkv, b_qkv, W_proj):
    """Host-side sharding: build the 8 per-core input dicts (bf16 casts)."""
    d = x.shape[-1]
    in_maps = []
    cosT = np.ascontiguousarray(cos.reshape(-1, HD).T).astype(np.float32)
    sinT = np.ascontiguousarray(sin.reshape(-1, HD).T).astype(np.float32)
    sinTs = sinT.copy()
    sinTs[: HD // 2] = -sinTs[: HD // 2]
    sinTr = np.roll(sinTs, -(HD // 2), axis=0)
    cosT = cosT.astype(BF)
    sinTr = sinTr.astype(BF)
    Wq = np.asarray(W_qkv[:, 0 * d:1 * d], np.float32)
    Wk = np.asarray(W_qkv[:, 1 * d:2 * d], np.float32)
    Wv = np.asarray(W_qkv[:, 2 * d:3 * d], np.float32)
    for c in range(N_CORES):
        b = c // 2
        g = c % 2
        hw = HPC * HD
        cs = slice(g * hw, (g + 1) * hw)
        in_maps.append(
            {
                "xT": np.ascontiguousarray(
                    np.asarray(x[b], np.float32).T).astype(BF),
                "wq": np.ascontiguousarray(Wq[:, cs]).astype(BF),
                "wk": np.ascontiguousarray(Wk[:, cs]).astype(BF),
                "wv": np.ascontiguousarray(Wv[:, cs]).astype(BF),
                "bq": np.ascontiguousarray(b_qkv[0 * d:1 * d][cs], np.float32),
                "bk": np.ascontiguousarray(b_qkv[1 * d:2 * d][cs], np.float32),
                "wp": np.ascontiguousarray(
                    np.asarray(W_proj, np.float32)[g * hw:(g + 1) * hw, :]
                ).astype(BF),
                "cosT": cosT,
                "sinTr": sinTr,
            }
        )
    return in_maps


def gather_output(results, b_qkv, W_proj, b_proj):
    """Sum the per-core partials and add the bias terms."""
    d = W_proj.shape[1]
    # v-bias contributes (sum_k attn = 1) exactly b_v @ W_proj per token.
    host_bias = (
        np.asarray(b_qkv[2 * d:3 * d], np.float32)
        @ np.asarray(W_proj, np.float32)
        + np.asarray(b_proj, np.float32)
    )
    y = np.empty((B, T, d), np.float32)
    for b in range(B):
        y[b] = results[2 * b]["y"] + results[2 * b + 1]["y"] + host_bias
    return y


_NC_CACHE = {}


def kernel(x, cos, sin, W_qkv, b_qkv, W_proj, b_proj):
    x = np.asarray(x, np.float32)
    key = "full"
    if key not in _NC_CACHE:
        _NC_CACHE[key] = build_nc()
    nc = _NC_CACHE[key]
    in_maps = make_in_maps(
        x,
        np.asarray(cos, np.float32),
        np.asarray(sin, np.float32),
        np.asarray(W_qkv, np.float32),
        np.asarray(b_qkv, np.float32),
        np.asarray(W_proj, np.float32),
    )
    res = run_bass_kernel_spmd(nc, in_maps, core_ids=list(range(N_CORES)))
    return gather_output(res.results, b_qkv, W_proj, b_proj)


if __name__ == "__main__":
    import reference

    inputs = reference.setup_inputs()
    out = kernel(**{k: np.asarray(v) for k, v in inputs.items()})
    exp = np.asarray(reference.reference(**inputs))
    err = np.abs(out - exp).max() / np.abs(exp).max()
    print("rel err:", err)
